# revision 77
# baseline (speedup 1.0000x reference)
"""CFConv (SchNet continuous-filter conv) Trainium2 Bass kernel, 8-core SPMD.

Reference computation:
    f    = x @ W_in                        # (40000, 128)
    f_j  = f[idx_j]                        # (640000, 128) gather
    wf   = w_ij * f_j                      # elementwise
    conv = segment_sum(wf, seg_i, 40000)   # seg_i sorted
    out  = conv @ W_out + b_out

Sharding: seg_i is sorted, so atoms are sharded into 8 contiguous ranges of
5000 and each core gets the contiguous run of edges whose seg_i falls in its
range (host searchsorted).  No collective: each core owns its output rows.

The device-side gather is eliminated entirely: f[idx_j] == x[idx_j] @ W_in,
and x[idx_j] is a pure row-permutation done on the host (same class of
layout transform as the w_ij re-bucketing).  The host uploads, per core, the
edge-ordered x_j (fp8 e3m4) and w_ij (bf16), bucketed by 128-atom
sub-window of seg_i and padded to a per-sub-window chunk capacity (max over
cores, so all 8 cores run one identical SPMD program).  Per chunk:

  mm1 (PE):  f_j[e,f]   = x_jT[k,e]^T @ W_in[k,f]        (-> PSUM f32)
  cpy (ACT): f_j PSUM f32 -> SBUF bf16, 4 chunks per instruction
  mul (DVE): wf[e,f]    = w[e,f] * f_j[e,f]              (all-bf16 SBUF: 2x mode)
  mm2 (PE):  convT[f,a] += wf[e,f]^T @ onehot[e,a]       (accum in PSUM)

The PSUM->SBUF staging hop runs on the otherwise-idle ACT engine 4/5 of
the time (a DVE multiply reading PSUM f32 directly pays a 120-cycle access
penalty and loses the 2-byte 2x_1p fast path); the remaining 1/5 multiply
straight from PSUM on DVE to balance the two engines.

Because seg_i is sorted, a 128-edge chunk spans at most ~21 atoms, so the
one-hot is a narrow 32-atom band per chunk (host-computed max-over-cores
base), built once per sub-window on DVE (is_equal against an iota, bf16 -
small integers are exact).  mm2s accumulate onto a PSUM bank initialized to
c (x) ones where c = b_out @ inv(W_out), folding the output bias into the
segment sum.  Per 512-atom window: convT -> bf16 (ACT), one fac2out matmul
outT[n,a] = W_out[f,n]^T @ convT[f,a] (N <= 512), and a contiguous DMA of
the transposed output.  The host transposes [128, 40000] back at the end.

Streams are fp8/bf16 against the 2e-2 relative harness gate (measured
1.25e-2, dominated by fp8 x_j; all-bf16 measures 4.3e-3 but costs ~32us
more DMA).  Measured ~127us/core: ~36 MB of input streams saturate all 16
DMA engines (~345 GB/s) for ~115us, with PE/ACT/DVE at 94/107/94us hidden
underneath.
"""

import numpy as np
import ml_dtypes

import concourse.bass as bass
import concourse.mybir as mybir
from concourse import bacc
from concourse.tile import TileContext

P = 128
NA = 40000          # atoms
NE = 640000         # edges
D = 128             # feature dim (FAN_IN == NFM == FAN_OUT)
NCORES = 8
APC = NA // NCORES  # atoms per core = 5000
WIN = 512           # atoms per PSUM window (1 bank)
SUB = 128           # atoms per sub-window (one-hot matmul N slice)
NSW = (APC + SUB - 1) // SUB   # sub-windows per core = 40
WPS = WIN // SUB    # sub-windows per window = 4
NWIN = (APC + WIN - 1) // WIN  # windows per core = 10

F32 = mybir.dt.float32
BF16 = mybir.dt.bfloat16
FP8 = mybir.dt.float8e3          # e3m4: 4 mantissa bits, range +-15.5
NPBF16 = ml_dtypes.bfloat16
NPFP8 = ml_dtypes.float8_e3m4
WBAND = 32          # one-hot band width (atoms); max observed chunk span = 21


def build_program(plan):
    """One SPMD program, identical across cores."""
    caps, abases = plan
    caps = [int(c) for c in caps]
    offs = [0]
    for c in caps:
        offs.append(offs[-1] + c)
    ctot = offs[-1]
    capmax = max(caps)

    nc = bacc.Bacc(None, target_bir_lowering=False, debug=False)

    xjdev_h = nc.dram_tensor("xjdev", [P, ctot * P], FP8, kind="ExternalInput")
    wdev_h = nc.dram_tensor("wdev", [P, ctot * P], BF16, kind="ExternalInput")
    segw_h = nc.dram_tensor("segw", [P, ctot], BF16, kind="ExternalInput")
    # iota3[p, c, aa] = aa, for the banded one-hot build
    iota_h = nc.dram_tensor("iota", [P, capmax * WBAND], BF16, kind="ExternalInput")
    win_h = nc.dram_tensor("Win", [P, P], BF16, kind="ExternalInput")
    wout_h = nc.dram_tensor("Wout", [P, P], BF16, kind="ExternalInput")
    # c = b_out @ inv(W_out): rank-1 PSUM init c (x) 1 replaces the bias add
    cvec_h = nc.dram_tensor("cvec", [1, P], BF16, kind="ExternalInput")
    out_h = nc.dram_tensor("out", [P, APC], F32, kind="ExternalOutput")

    GRP = 4    # chunks per mm1 PSUM group (one 2KB bank)
    LOOKG = 3  # mm1 groups in flight ahead of the copy/mul/mm2 tail

    with TileContext(nc) as tc:
        with tc.tile_pool(name="const", bufs=1) as const:
            win_t = const.tile([P, P], BF16)
            nc.sync.dma_start(win_t[:], win_h[:, :])
            wout_t = const.tile([P, P], BF16)
            nc.sync.dma_start(wout_t[:], wout_h[:, :])
            cvec_t = const.tile([1, P], BF16)
            nc.sync.dma_start(cvec_t[:], cvec_h[:, :])
            iota_t = const.tile([P, capmax, WBAND], BF16)
            nc.sync.dma_start(
                iota_t[:], iota_h[:, :].rearrange("p (c a) -> p c a", a=WBAND)
            )
            segw_t = const.tile([P, ctot], BF16)
            nc.sync.dma_start(segw_t[:], segw_h[:, :])
            ones_t = const.tile([1, WIN], BF16)
            nc.gpsimd.memset(ones_t[:], 1.0)

            with (
                tc.tile_pool(name="xjp", bufs=4) as xjp,
                tc.tile_pool(name="wp", bufs=4) as wp,
                tc.tile_pool(name="ohp", bufs=4) as ohp,
                tc.tile_pool(name="wfp", bufs=6) as wfp,
                tc.tile_pool(name="fjp", bufs=4) as fjp,
                tc.tile_pool(name="cvp", bufs=2) as cvp,
                tc.tile_pool(name="owp", bufs=2) as owp,
                tc.tile_pool(name="ps1", bufs=LOOKG + 2, space="PSUM") as ps1,
                tc.tile_pool(name="ps2", bufs=2, space="PSUM") as ps2,
                tc.tile_pool(name="ps3", bufs=1, space="PSUM") as ps3,
            ):
                psT = None
                pending = None  # deferred fac2out for the finished window

                def flush_pending():
                    nonlocal pending
                    if pending is None:
                        return
                    fin_psT, wa0, wan = pending
                    pending = None
                    cvt = cvp.tile([P, WIN], BF16)
                    nc.scalar.copy(cvt[:, :wan], fin_psT[:, :wan])
                    ops3 = ps3.tile([P, WIN], F32)
                    nc.tensor.matmul(
                        ops3[:, :wan],
                        lhsT=wout_t[:],
                        rhs=cvt[:, :wan],
                        start=True,
                        stop=True,
                    )
                    # bias already folded in via the cvec PSUM init; the
                    # staging copy runs on DVE to keep ACT for fjs copies
                    ow = owp.tile([P, WIN], F32)
                    nc.vector.tensor_copy(ow[:, :wan], ops3[:, :wan])
                    nc.scalar.dma_start(out_h[:, wa0 : wa0 + wan], ow[:, :wan])

                gctr = 0  # global group counter for multiply-engine routing
                for s in range(NSW):
                    w_i, sl = divmod(s, WPS)
                    cap = caps[s]
                    off = offs[s]
                    ab = abases[s]
                    xjt = xjp.tile([P, cap, P], FP8)
                    nc.sync.dma_start(
                        xjt[:], xjdev_h[:, off * P : (off + cap) * P].rearrange(
                            "p (c e) -> p c e", e=P
                        )
                    )
                    wt = wp.tile([P, cap, P], BF16)
                    # separate HWDGE queue: halves the per-queue descriptor
                    # supply latency and avoids xjt/wt head-of-line blocking
                    nc.gpsimd.dma_start(
                        wt[:], wdev_h[:, off * P : (off + cap) * P].rearrange(
                            "p (c e) -> p c e", e=P
                        )
                    )
                    # banded one-hot (gpsimd/ACT cannot run TensorTensor, so
                    # DVE it is; the narrow band keeps the 1x cost small);
                    # [P(edge), chunk, atom] stays contiguous for the mm2 rhs
                    oh = ohp.tile([P, cap, WBAND], BF16)
                    nc.vector.tensor_tensor(
                        out=oh[:],
                        in0=segw_t[:, off : off + cap]
                        .unsqueeze(2)
                        .to_broadcast([P, cap, WBAND]),
                        in1=iota_t[:, :cap, :],
                        op=mybir.AluOpType.is_equal,
                    )
                    if sl == 0:
                        psT = ps2.tile([P, WIN], F32)
                        # init the bank to c (x) ones (bias folded through
                        # inv(W_out)); mm2s accumulate (start=False) since
                        # neighboring bands overlap
                        nc.tensor.matmul(
                            psT[:], lhsT=cvec_t[:, :], rhs=ones_t[:, :],
                            start=True, stop=True, skip_group_check=True,
                        )

                    ngrp = (cap + GRP - 1) // GRP
                    grp_ps = {}

                    def emit_m1g(g):
                        r = min(GRP, cap - g * GRP)
                        fj = ps1.tile([P, GRP, P], F32)
                        for i in range(r):
                            nc.tensor.matmul(
                                fj[:, i, :],
                                lhsT=xjt[:, g * GRP + i, :],
                                rhs=win_t[:],
                                start=True,
                                stop=True,
                            )
                        grp_ps[g] = (fj, r)

                    def emit_tail(g):
                        nonlocal gctr
                        fj, r = grp_ps.pop(g)
                        c0 = g * GRP
                        # per-group wf tile: the next group's multiply must
                        # not inherit a WAR dependency on this group's mm2s
                        wf = wfp.tile([P, GRP, P], BF16)
                        # 4/5 of multiplies stage through ACT for the DVE 2x
                        # path; 1/5 read PSUM directly on DVE (balances the
                        # ACT and DVE engine budgets)
                        route = (0, 0, 0, 0, 1)[gctr % 5]
                        gctr += 1
                        if route == 0:
                            fjs = fjp.tile([P, GRP, P], BF16)
                            nc.scalar.copy(fjs[:, :r, :], fj[:, :r, :])
                            nc.vector.tensor_mul(
                                wf[:, :r, :],
                                wt[:, c0 : c0 + r, :],
                                fjs[:, :r, :],
                            )
                        else:
                            nc.vector.tensor_mul(
                                wf[:, :r, :],
                                wt[:, c0 : c0 + r, :],
                                fj[:, :r, :],
                            )
                        for i in range(r):
                            ch = c0 + i
                            a0 = sl * SUB + ab[ch]
                            nc.tensor.matmul(
                                psT[:, a0 : a0 + WBAND],
                                lhsT=wf[:, i, :],
                                rhs=oh[:, ch, :],
                                start=False,
                                stop=True,
                                skip_group_check=True,
                            )

                    for g in range(min(LOOKG, ngrp)):
                        emit_m1g(g)
                    # flush fac2out one sub-window AFTER the window closes:
                    # by then its mm2s have retired, so the ACT-queue cvt
                    # doesn't head-of-line-block the fjs copies behind it
                    if sl == 1 or s == NSW - 1:
                        flush_pending()
                    for g in range(ngrp):
                        if g + LOOKG < ngrp:
                            emit_m1g(g + LOOKG)
                        emit_tail(g)

                    if sl == WPS - 1 or s == NSW - 1:
                        wa0 = w_i * WIN
                        pending = (psT, wa0, min(WIN, APC - wa0))
                flush_pending()
    return nc


def prepare(inputs):
    """Host-side sharding: per-core padded edge buckets in bf16."""
    x = np.ascontiguousarray(np.asarray(inputs["x"], dtype=np.float32))
    w_ij = np.ascontiguousarray(np.asarray(inputs["w_ij"], dtype=np.float32))
    seg_i = np.asarray(inputs["seg_i"]).astype(np.int64).ravel()
    idx_j = np.asarray(inputs["idx_j"]).astype(np.int64).ravel()
    W_in = np.asarray(inputs["W_in"], dtype=np.float32)
    W_out = np.asarray(inputs["W_out"], dtype=np.float32)
    b_out = np.asarray(inputs["b_out"], dtype=np.float32).ravel()

    # edge run boundaries for every 128-atom sub-window of every core
    bounds = np.asarray(
        [c * APC + s * SUB for c in range(NCORES) for s in range(NSW)] + [NA],
        dtype=np.int64,
    )
    edges = np.searchsorted(seg_i, bounds)
    n = (edges[1:] - edges[:-1]).reshape(NCORES, NSW)
    caps = np.maximum(1, -(-n.max(axis=0) // P))  # per-sub-window chunk cap
    offs = np.concatenate([[0], np.cumsum(caps)])
    ctot = int(offs[-1])
    capmax = int(caps.max())

    x_f8 = x.astype(NPFP8)
    w_bf = w_ij.astype(NPBF16)
    # per-(s, ch) narrow-band atom base: union of the chunk's atom range
    # over all 8 cores (seg_i sorted => span is small; measured max 21)
    abases = []
    for s in range(NSW):
        cap = int(caps[s])
        ab = []
        for ch in range(cap):
            lo_u, hi_u = P, -1
            for c in range(NCORES):
                l, h = int(edges[c * NSW + s]), int(edges[c * NSW + s + 1])
                chunk = seg_i[l + ch * P : l + min((ch + 1) * P, h - l)]
                if chunk.size:
                    base = c * APC + s * SUB
                    lo_u = min(lo_u, int(chunk[0] - base))
                    hi_u = max(hi_u, int(chunk[-1] - base))
            if hi_u < 0:
                ab.append(0)
            else:
                assert hi_u - lo_u + 1 <= WBAND, "chunk atom span exceeds WBAND"
                ab.append(max(0, min(lo_u, P - WBAND)))
        abases.append(ab)

    # iota3[p, c, aa] = aa, flattened to [P, capmax*WBAND]
    iota_t = np.ascontiguousarray(
        np.broadcast_to(
            np.tile(np.arange(WBAND, dtype=np.float32), capmax).astype(NPBF16),
            (P, capmax * WBAND),
        )
    )
    win_b = W_in.astype(NPBF16)
    wout_b = W_out.astype(NPBF16)
    # bias folded through inv(W_out): psT init with c makes conv@W_out
    # come out pre-biased
    cvec = np.linalg.solve(
        W_out.astype(np.float64).T, b_out.astype(np.float64)
    ).astype(np.float32)[None, :].astype(NPBF16)

    in_maps = []
    for c in range(NCORES):
        xjdev = np.zeros((P, ctot * P), dtype=NPFP8)
        wdev = np.zeros((P, ctot * P), dtype=NPBF16)
        segw = np.full((P, ctot), -1.0, dtype=NPBF16)
        for s in range(NSW):
            k = c * NSW + s
            lo, hi = int(edges[k]), int(edges[k + 1])
            cnt = hi - lo
            cap = int(caps[s])
            off = int(offs[s])
            xj = np.zeros((cap * P, D), dtype=NPFP8)
            xj[:cnt] = x_f8[idx_j[lo:hi]]
            # lhsT layout [k, (chunk, edge)]
            xjdev[:, off * P : (off + cap) * P] = (
                xj.reshape(cap, P, D).transpose(2, 0, 1).reshape(D, cap * P)
            )
            wpad = np.zeros((cap * P, D), dtype=NPBF16)
            wpad[:cnt] = w_bf[lo:hi]
            # [edge, (chunk, feature)]
            wdev[:, off * P : (off + cap) * P] = (
                wpad.reshape(cap, P, D).transpose(1, 0, 2).reshape(P, cap * P)
            )
            sp = np.full(cap * P, -1.0, dtype=np.float32)
            sp[:cnt] = (seg_i[lo:hi] - (c * APC + s * SUB)).astype(np.float32)
            # rebase each chunk to its narrow-band window
            for ch in range(cap):
                sp[ch * P : (ch + 1) * P] -= np.float32(abases[s][ch])
            sp[cnt:] = -1.0
            segw[:, off : off + cap] = sp.reshape(cap, P).T.astype(NPBF16)
        in_maps.append(
            {
                "xjdev": xjdev,
                "wdev": wdev,
                "segw": segw,
                "iota": iota_t,
                "Win": win_b,
                "Wout": wout_b,
                "cvec": cvec,
            }
        )
    return ([int(c) for c in caps], abases), in_maps


def kernel(**inputs) -> np.ndarray:
    from concourse.bass_utils import run_bass_kernel_spmd

    plan, in_maps = prepare(inputs)
    nc = build_program(plan)
    nc.finalize()
    res = run_bass_kernel_spmd(nc, in_maps, core_ids=list(range(NCORES)))
    outT = np.concatenate([r["out"] for r in res.results], axis=1)
    return np.ascontiguousarray(outT.T)


# revision 79
# speedup vs baseline: 1.0514x; 1.0514x over previous
"""CFConv (SchNet continuous-filter conv) Trainium2 Bass kernel, 8-core SPMD.

Reference computation:
    f    = x @ W_in                        # (40000, 128)
    f_j  = f[idx_j]                        # (640000, 128) gather
    wf   = w_ij * f_j                      # elementwise
    conv = segment_sum(wf, seg_i, 40000)   # seg_i sorted
    out  = conv @ W_out + b_out

Sharding: seg_i is sorted, so atoms are sharded into 8 contiguous ranges of
5000 and each core gets the contiguous run of edges whose seg_i falls in its
range (host searchsorted).  No collective: each core owns its output rows.

The device-side gather is eliminated entirely: f[idx_j] == x[idx_j] @ W_in,
and x[idx_j] is a pure row-permutation done on the host (same class of
layout transform as the w_ij re-bucketing).  The host uploads, per core, the
edge-ordered x_j (fp8 e3m4) and w_ij (bf16), bucketed by 128-atom
sub-window of seg_i and padded to a per-sub-window chunk capacity (max over
cores, so all 8 cores run one identical SPMD program).  Per chunk:

  mm1 (PE):  f_j[e,f]   = x_jT[k,e]^T @ W_in[k,f]        (-> PSUM f32)
  cpy (ACT): f_j PSUM f32 -> SBUF bf16, 4 chunks per instruction
  mul (DVE): wf[e,f]    = w[e,f] * f_j[e,f]              (all-bf16 SBUF: 2x mode)
  mm2 (PE):  convT[f,a] += wf[e,f]^T @ onehot[e,a]       (accum in PSUM)

The PSUM->SBUF staging hop runs on the otherwise-idle ACT engine 4/5 of
the time (a DVE multiply reading PSUM f32 directly pays a 120-cycle access
penalty and loses the 2-byte 2x_1p fast path); the remaining 1/5 multiply
straight from PSUM on DVE to balance the two engines.

Because seg_i is sorted, a 128-edge chunk spans at most ~21 atoms, so the
one-hot is a narrow 32-atom band per chunk (host-computed max-over-cores
base), built once per sub-window on DVE (is_equal against an iota, bf16 -
small integers are exact).  mm2s accumulate onto a PSUM bank initialized to
c (x) ones where c = b_out @ inv(W_out), folding the output bias into the
segment sum.  Per 512-atom window: convT -> bf16 (ACT), one fac2out matmul
outT[n,a] = W_out[f,n]^T @ convT[f,a] (N <= 512), and a contiguous DMA of
the transposed output.  The host transposes [128, 40000] back at the end.

Streams are fp8/bf16 against the 2e-2 relative harness gate (measured
1.25e-2, dominated by fp8 x_j; all-bf16 measures 4.3e-3 but costs ~32us
more DMA).  Measured ~127us/core: ~36 MB of input streams saturate all 16
DMA engines (~345 GB/s) for ~115us, with PE/ACT/DVE at 94/107/94us hidden
underneath.
"""

import numpy as np
import ml_dtypes

import concourse.bass as bass
import concourse.mybir as mybir
from concourse import bacc
from concourse.tile import TileContext

P = 128
NA = 40000          # atoms
NE = 640000         # edges
D = 128             # feature dim (FAN_IN == NFM == FAN_OUT)
NCORES = 8
APC = NA // NCORES  # atoms per core = 5000
WIN = 512           # atoms per PSUM window (1 bank)
SUB = 128           # atoms per sub-window (one-hot matmul N slice)
NSW = (APC + SUB - 1) // SUB   # sub-windows per core = 40
WPS = WIN // SUB    # sub-windows per window = 4
NWIN = (APC + WIN - 1) // WIN  # windows per core = 10

F32 = mybir.dt.float32
BF16 = mybir.dt.bfloat16
FP8 = mybir.dt.float8e3          # e3m4: 4 mantissa bits, range +-15.5
NPBF16 = ml_dtypes.bfloat16
NPFP8 = ml_dtypes.float8_e3m4
WBAND = 32          # one-hot band width (atoms); max observed chunk span = 21


def build_program(plan):
    """One SPMD program, identical across cores."""
    caps, abases = plan
    caps = [int(c) for c in caps]
    offs = [0]
    for c in caps:
        offs.append(offs[-1] + c)
    ctot = offs[-1]
    capmax = max(caps)

    nc = bacc.Bacc(None, target_bir_lowering=False, debug=False)

    xjdev_h = nc.dram_tensor("xjdev", [P, ctot * P], FP8, kind="ExternalInput")
    wdev_h = nc.dram_tensor("wdev", [P, ctot * P], BF16, kind="ExternalInput")
    segw_h = nc.dram_tensor("segw", [P, ctot], BF16, kind="ExternalInput")
    # iota3[p, c, aa] = aa, for the banded one-hot build
    iota_h = nc.dram_tensor("iota", [P, capmax * WBAND], BF16, kind="ExternalInput")
    win_h = nc.dram_tensor("Win", [P, P], BF16, kind="ExternalInput")
    wout_h = nc.dram_tensor("Wout", [P, P], BF16, kind="ExternalInput")
    # c = b_out @ inv(W_out): rank-1 PSUM init c (x) 1 replaces the bias add
    cvec_h = nc.dram_tensor("cvec", [1, P], BF16, kind="ExternalInput")
    out_h = nc.dram_tensor("out", [P, APC], F32, kind="ExternalOutput")

    GRP = 4    # chunks per mm1 PSUM group (one 2KB bank)
    LOOKG = 3  # mm1 groups in flight ahead of the copy/mul/mm2 tail

    with TileContext(nc) as tc:
        with tc.tile_pool(name="const", bufs=1) as const:
            win_t = const.tile([P, P], BF16)
            nc.sync.dma_start(win_t[:], win_h[:, :])
            wout_t = const.tile([P, P], BF16)
            nc.sync.dma_start(wout_t[:], wout_h[:, :])
            cvec_t = const.tile([1, P], BF16)
            nc.sync.dma_start(cvec_t[:], cvec_h[:, :])
            iota_t = const.tile([P, capmax, WBAND], BF16)
            nc.sync.dma_start(
                iota_t[:], iota_h[:, :].rearrange("p (c a) -> p c a", a=WBAND)
            )
            segw_t = const.tile([P, ctot], BF16)
            nc.sync.dma_start(segw_t[:], segw_h[:, :])
            ones_t = const.tile([1, WIN], BF16)
            nc.gpsimd.memset(ones_t[:], 1.0)

            with (
                tc.tile_pool(name="xjp", bufs=4) as xjp,
                tc.tile_pool(name="wp", bufs=4) as wp,
                tc.tile_pool(name="ohp", bufs=4) as ohp,
                tc.tile_pool(name="wfp", bufs=6) as wfp,
                tc.tile_pool(name="fjp", bufs=4) as fjp,
                tc.tile_pool(name="cvp", bufs=2) as cvp,
                tc.tile_pool(name="owp", bufs=2) as owp,
                tc.tile_pool(name="ps1", bufs=LOOKG + 2, space="PSUM") as ps1,
                tc.tile_pool(name="ps2", bufs=2, space="PSUM") as ps2,
                tc.tile_pool(name="ps3", bufs=1, space="PSUM") as ps3,
            ):
                psT = None
                pending = None  # deferred fac2out for the finished window

                def flush_pending():
                    nonlocal pending
                    if pending is None:
                        return
                    fin_psT, wa0, wan = pending
                    pending = None
                    cvt = cvp.tile([P, WIN], BF16)
                    nc.scalar.copy(cvt[:, :wan], fin_psT[:, :wan])
                    ops3 = ps3.tile([P, WIN], F32)
                    nc.tensor.matmul(
                        ops3[:, :wan],
                        lhsT=wout_t[:],
                        rhs=cvt[:, :wan],
                        start=True,
                        stop=True,
                    )
                    # bias already folded in via the cvec PSUM init
                    ow = owp.tile([P, WIN], F32)
                    nc.scalar.copy(ow[:, :wan], ops3[:, :wan])
                    nc.scalar.dma_start(out_h[:, wa0 : wa0 + wan], ow[:, :wan])

                gctr = 0  # global group counter for multiply-engine routing
                for s in range(NSW):
                    w_i, sl = divmod(s, WPS)
                    cap = caps[s]
                    off = offs[s]
                    ab = abases[s]
                    xjt = xjp.tile([P, cap, P], FP8)
                    nc.sync.dma_start(
                        xjt[:], xjdev_h[:, off * P : (off + cap) * P].rearrange(
                            "p (c e) -> p c e", e=P
                        )
                    )
                    wt = wp.tile([P, cap, P], BF16)
                    nc.sync.dma_start(
                        wt[:], wdev_h[:, off * P : (off + cap) * P].rearrange(
                            "p (c e) -> p c e", e=P
                        )
                    )
                    # banded one-hot (gpsimd/ACT cannot run TensorTensor, so
                    # DVE it is; the narrow band keeps the 1x cost small);
                    # [P(edge), chunk, atom] stays contiguous for the mm2 rhs
                    oh = ohp.tile([P, cap, WBAND], BF16)
                    nc.vector.tensor_tensor(
                        out=oh[:],
                        in0=segw_t[:, off : off + cap]
                        .unsqueeze(2)
                        .to_broadcast([P, cap, WBAND]),
                        in1=iota_t[:, :cap, :],
                        op=mybir.AluOpType.is_equal,
                    )
                    if sl == 0:
                        psT = ps2.tile([P, WIN], F32)
                        # init the bank to c (x) ones (bias folded through
                        # inv(W_out)); mm2s accumulate (start=False) since
                        # neighboring bands overlap
                        nc.tensor.matmul(
                            psT[:], lhsT=cvec_t[:, :], rhs=ones_t[:, :],
                            start=True, stop=True, skip_group_check=True,
                        )

                    ngrp = (cap + GRP - 1) // GRP
                    grp_ps = {}

                    def emit_m1g(g):
                        r = min(GRP, cap - g * GRP)
                        fj = ps1.tile([P, GRP, P], F32)
                        for i in range(r):
                            nc.tensor.matmul(
                                fj[:, i, :],
                                lhsT=xjt[:, g * GRP + i, :],
                                rhs=win_t[:],
                                start=True,
                                stop=True,
                            )
                        grp_ps[g] = (fj, r)

                    def emit_tail(g):
                        nonlocal gctr
                        fj, r = grp_ps.pop(g)
                        c0 = g * GRP
                        # per-group wf tile: the next group's multiply must
                        # not inherit a WAR dependency on this group's mm2s
                        wf = wfp.tile([P, GRP, P], BF16)
                        # 4/5 of multiplies stage through ACT for the DVE 2x
                        # path; 1/5 read PSUM directly on DVE (balances the
                        # ACT and DVE engine budgets)
                        route = (0, 0, 0, 0, 1)[gctr % 5]
                        gctr += 1
                        if route == 0:
                            fjs = fjp.tile([P, GRP, P], BF16)
                            nc.scalar.copy(fjs[:, :r, :], fj[:, :r, :])
                            nc.vector.tensor_mul(
                                wf[:, :r, :],
                                wt[:, c0 : c0 + r, :],
                                fjs[:, :r, :],
                            )
                        else:
                            nc.vector.tensor_mul(
                                wf[:, :r, :],
                                wt[:, c0 : c0 + r, :],
                                fj[:, :r, :],
                            )
                        for i in range(r):
                            ch = c0 + i
                            a0 = sl * SUB + ab[ch]
                            nc.tensor.matmul(
                                psT[:, a0 : a0 + WBAND],
                                lhsT=wf[:, i, :],
                                rhs=oh[:, ch, :],
                                start=False,
                                stop=True,
                                skip_group_check=True,
                            )

                    for g in range(min(LOOKG, ngrp)):
                        emit_m1g(g)
                    # flush fac2out one sub-window AFTER the window closes:
                    # by then its mm2s have retired, so the ACT-queue cvt
                    # doesn't head-of-line-block the fjs copies behind it
                    if sl == 1 or s == NSW - 1:
                        flush_pending()
                    for g in range(ngrp):
                        if g + LOOKG < ngrp:
                            emit_m1g(g + LOOKG)
                        emit_tail(g)

                    if sl == WPS - 1 or s == NSW - 1:
                        wa0 = w_i * WIN
                        pending = (psT, wa0, min(WIN, APC - wa0))
                flush_pending()
    return nc


def prepare(inputs):
    """Host-side sharding: per-core padded edge buckets in bf16."""
    x = np.ascontiguousarray(np.asarray(inputs["x"], dtype=np.float32))
    w_ij = np.ascontiguousarray(np.asarray(inputs["w_ij"], dtype=np.float32))
    seg_i = np.asarray(inputs["seg_i"]).astype(np.int64).ravel()
    idx_j = np.asarray(inputs["idx_j"]).astype(np.int64).ravel()
    W_in = np.asarray(inputs["W_in"], dtype=np.float32)
    W_out = np.asarray(inputs["W_out"], dtype=np.float32)
    b_out = np.asarray(inputs["b_out"], dtype=np.float32).ravel()

    # edge run boundaries for every 128-atom sub-window of every core
    bounds = np.asarray(
        [c * APC + s * SUB for c in range(NCORES) for s in range(NSW)] + [NA],
        dtype=np.int64,
    )
    edges = np.searchsorted(seg_i, bounds)
    n = (edges[1:] - edges[:-1]).reshape(NCORES, NSW)
    caps = np.maximum(1, -(-n.max(axis=0) // P))  # per-sub-window chunk cap
    offs = np.concatenate([[0], np.cumsum(caps)])
    ctot = int(offs[-1])
    capmax = int(caps.max())

    x_f8 = x.astype(NPFP8)
    w_bf = w_ij.astype(NPBF16)
    # per-(s, ch) narrow-band atom base: union of the chunk's atom range
    # over all 8 cores (seg_i sorted => span is small; measured max 21)
    abases = []
    for s in range(NSW):
        cap = int(caps[s])
        ab = []
        for ch in range(cap):
            lo_u, hi_u = P, -1
            for c in range(NCORES):
                l, h = int(edges[c * NSW + s]), int(edges[c * NSW + s + 1])
                chunk = seg_i[l + ch * P : l + min((ch + 1) * P, h - l)]
                if chunk.size:
                    base = c * APC + s * SUB
                    lo_u = min(lo_u, int(chunk[0] - base))
                    hi_u = max(hi_u, int(chunk[-1] - base))
            if hi_u < 0:
                ab.append(0)
            else:
                assert hi_u - lo_u + 1 <= WBAND, "chunk atom span exceeds WBAND"
                ab.append(max(0, min(lo_u, P - WBAND)))
        abases.append(ab)

    # iota3[p, c, aa] = aa, flattened to [P, capmax*WBAND]
    iota_t = np.ascontiguousarray(
        np.broadcast_to(
            np.tile(np.arange(WBAND, dtype=np.float32), capmax).astype(NPBF16),
            (P, capmax * WBAND),
        )
    )
    win_b = W_in.astype(NPBF16)
    wout_b = W_out.astype(NPBF16)
    # bias folded through inv(W_out): psT init with c makes conv@W_out
    # come out pre-biased
    cvec = np.linalg.solve(
        W_out.astype(np.float64).T, b_out.astype(np.float64)
    ).astype(np.float32)[None, :].astype(NPBF16)

    in_maps = []
    for c in range(NCORES):
        xjdev = np.zeros((P, ctot * P), dtype=NPFP8)
        wdev = np.zeros((P, ctot * P), dtype=NPBF16)
        segw = np.full((P, ctot), -1.0, dtype=NPBF16)
        for s in range(NSW):
            k = c * NSW + s
            lo, hi = int(edges[k]), int(edges[k + 1])
            cnt = hi - lo
            cap = int(caps[s])
            off = int(offs[s])
            xj = np.zeros((cap * P, D), dtype=NPFP8)
            xj[:cnt] = x_f8[idx_j[lo:hi]]
            # lhsT layout [k, (chunk, edge)]
            xjdev[:, off * P : (off + cap) * P] = (
                xj.reshape(cap, P, D).transpose(2, 0, 1).reshape(D, cap * P)
            )
            wpad = np.zeros((cap * P, D), dtype=NPBF16)
            wpad[:cnt] = w_bf[lo:hi]
            # [edge, (chunk, feature)]
            wdev[:, off * P : (off + cap) * P] = (
                wpad.reshape(cap, P, D).transpose(1, 0, 2).reshape(P, cap * P)
            )
            sp = np.full(cap * P, -1.0, dtype=np.float32)
            sp[:cnt] = (seg_i[lo:hi] - (c * APC + s * SUB)).astype(np.float32)
            # rebase each chunk to its narrow-band window
            for ch in range(cap):
                sp[ch * P : (ch + 1) * P] -= np.float32(abases[s][ch])
            sp[cnt:] = -1.0
            segw[:, off : off + cap] = sp.reshape(cap, P).T.astype(NPBF16)
        in_maps.append(
            {
                "xjdev": xjdev,
                "wdev": wdev,
                "segw": segw,
                "iota": iota_t,
                "Win": win_b,
                "Wout": wout_b,
                "cvec": cvec,
            }
        )
    return ([int(c) for c in caps], abases), in_maps


def kernel(**inputs) -> np.ndarray:
    from concourse.bass_utils import run_bass_kernel_spmd

    plan, in_maps = prepare(inputs)
    nc = build_program(plan)
    nc.finalize()
    res = run_bass_kernel_spmd(nc, in_maps, core_ids=list(range(NCORES)))
    outT = np.concatenate([r["out"] for r in res.results], axis=1)
    return np.ascontiguousarray(outT.T)


# revision 87
# speedup vs baseline: 1.0768x; 1.0242x over previous
"""CFConv (SchNet continuous-filter conv) Trainium2 Bass kernel, 8-core SPMD.

Reference computation:
    f    = x @ W_in                        # (40000, 128)
    f_j  = f[idx_j]                        # (640000, 128) gather
    wf   = w_ij * f_j                      # elementwise
    conv = segment_sum(wf, seg_i, 40000)   # seg_i sorted
    out  = conv @ W_out + b_out

Sharding: seg_i is sorted, so atoms are sharded into 8 contiguous ranges of
5000 and each core gets the contiguous run of edges whose seg_i falls in its
range (host searchsorted).  No collective: each core owns its output rows.

The device-side gather is eliminated entirely: f[idx_j] == x[idx_j] @ W_in,
and x[idx_j] is a pure row-permutation done on the host (same class of
layout transform as the w_ij re-bucketing).  The host uploads, per core, the
edge-ordered x_j (fp8 e3m4) and w_ij (bf16), bucketed by 128-atom
sub-window of seg_i and padded to a per-sub-window chunk capacity (max over
cores, so all 8 cores run one identical SPMD program).  Per chunk:

  mm1 (PE):  f_j[e,f]   = x_jT[k,e]^T @ W_in[k,f]        (-> PSUM f32)
  cpy (ACT): f_j PSUM f32 -> SBUF bf16, 4 chunks per instruction
  mul (DVE): wf[e,f]    = w[e,f] * f_j[e,f]              (all-bf16 SBUF: 2x mode)
  mm2 (PE):  convT[f,a] += wf[e,f]^T @ onehot[e,a]       (accum in PSUM)

The PSUM->SBUF staging hop runs on the otherwise-idle ACT engine 4/5 of
the time (a DVE multiply reading PSUM f32 directly pays a 120-cycle access
penalty and loses the 2-byte 2x_1p fast path); the remaining 1/5 multiply
straight from PSUM on DVE to balance the two engines.

Because seg_i is sorted, a 128-edge chunk spans at most ~21 atoms, so the
one-hot is a narrow 32-atom band per chunk (host-computed max-over-cores
base), built once per sub-window on DVE (is_equal against an iota, bf16 -
small integers are exact).  mm2s accumulate onto a PSUM bank initialized to
c (x) ones where c = b_out @ inv(W_out), folding the output bias into the
segment sum.  Per 512-atom window: convT -> bf16 (ACT), one fac2out matmul
outT[n,a] = W_out[f,n]^T @ convT[f,a] (N <= 512), and a contiguous DMA of
the transposed output.  The host transposes [128, 40000] back at the end.

Streams are fp8/bf16 against the 2e-2 relative harness gate (measured
1.25e-2, dominated by fp8 x_j; all-bf16 measures 4.3e-3 but costs ~32us
more DMA).  Measured ~127us/core: ~36 MB of input streams saturate all 16
DMA engines (~345 GB/s) for ~115us, with PE/ACT/DVE at 94/107/94us hidden
underneath.
"""

import numpy as np
import ml_dtypes

import concourse.bass as bass
import concourse.mybir as mybir
from concourse import bacc
from concourse.tile import TileContext

P = 128
NA = 40000          # atoms
NE = 640000         # edges
D = 128             # feature dim (FAN_IN == NFM == FAN_OUT)
NCORES = 8
APC = NA // NCORES  # atoms per core = 5000
WIN = 512           # atoms per PSUM window (1 bank)
SUB = 128           # atoms per sub-window (one-hot matmul N slice)
NSW = (APC + SUB - 1) // SUB   # sub-windows per core = 40
WPS = WIN // SUB    # sub-windows per window = 4
NWIN = (APC + WIN - 1) // WIN  # windows per core = 10

F32 = mybir.dt.float32
BF16 = mybir.dt.bfloat16
FP8 = mybir.dt.float8e3          # e3m4: 4 mantissa bits, range +-15.5
NPBF16 = ml_dtypes.bfloat16
NPFP8 = ml_dtypes.float8_e3m4
WBAND = 32          # one-hot band width (atoms); max observed chunk span = 21


def build_program(plan):
    """One SPMD program, identical across cores."""
    caps, abases = plan
    caps = [int(c) for c in caps]
    offs = [0]
    for c in caps:
        offs.append(offs[-1] + c)
    ctot = offs[-1]
    capmax = max(caps)

    nc = bacc.Bacc(None, target_bir_lowering=False, debug=False)

    xjdev_h = nc.dram_tensor("xjdev", [P, ctot * P], FP8, kind="ExternalInput")
    wdev_h = nc.dram_tensor("wdev", [P, ctot * P], BF16, kind="ExternalInput")
    segw_h = nc.dram_tensor("segw", [P, ctot], BF16, kind="ExternalInput")
    # iota3[p, c, aa] = aa, for the banded one-hot build
    iota_h = nc.dram_tensor("iota", [P, capmax * WBAND], BF16, kind="ExternalInput")
    win_h = nc.dram_tensor("Win", [P, P], BF16, kind="ExternalInput")
    wout_h = nc.dram_tensor("Wout", [P, P], BF16, kind="ExternalInput")
    # c = b_out @ inv(W_out): rank-1 PSUM init c (x) 1 replaces the bias add
    cvec_h = nc.dram_tensor("cvec", [1, P], BF16, kind="ExternalInput")
    out_h = nc.dram_tensor("out", [P, APC], F32, kind="ExternalOutput")

    GRP = 4    # chunks per mm1 PSUM group (one 2KB bank)
    LOOKG = 3  # mm1 groups in flight ahead of the copy/mul/mm2 tail

    with TileContext(nc) as tc:
        with tc.tile_pool(name="const", bufs=1) as const:
            win_t = const.tile([P, P], BF16)
            nc.sync.dma_start(win_t[:], win_h[:, :])
            wout_t = const.tile([P, P], BF16)
            nc.sync.dma_start(wout_t[:], wout_h[:, :])
            cvec_t = const.tile([1, P], BF16)
            nc.sync.dma_start(cvec_t[:], cvec_h[:, :])
            iota_t = const.tile([P, capmax, WBAND], BF16)
            nc.sync.dma_start(
                iota_t[:], iota_h[:, :].rearrange("p (c a) -> p c a", a=WBAND)
            )
            segw_t = const.tile([P, ctot], BF16)
            nc.sync.dma_start(segw_t[:], segw_h[:, :])
            ones_t = const.tile([1, WIN], BF16)
            nc.gpsimd.memset(ones_t[:], 1.0)

            with (
                tc.tile_pool(name="xjp", bufs=6) as xjp,
                tc.tile_pool(name="wp", bufs=6) as wp,
                tc.tile_pool(name="ohp", bufs=4) as ohp,
                tc.tile_pool(name="wfp", bufs=6) as wfp,
                tc.tile_pool(name="fjp", bufs=4) as fjp,
                tc.tile_pool(name="cvp", bufs=2) as cvp,
                tc.tile_pool(name="owp", bufs=2) as owp,
                tc.tile_pool(name="ps1", bufs=LOOKG + 2, space="PSUM") as ps1,
                tc.tile_pool(name="ps2", bufs=2, space="PSUM") as ps2,
                tc.tile_pool(name="ps3", bufs=1, space="PSUM") as ps3,
            ):
                psT = None
                pending = None  # deferred fac2out for the finished window

                def flush_pending():
                    nonlocal pending
                    if pending is None:
                        return
                    fin_psT, wa0, wan = pending
                    pending = None
                    cvt = cvp.tile([P, WIN], BF16)
                    nc.scalar.copy(cvt[:, :wan], fin_psT[:, :wan])
                    ops3 = ps3.tile([P, WIN], F32)
                    nc.tensor.matmul(
                        ops3[:, :wan],
                        lhsT=wout_t[:],
                        rhs=cvt[:, :wan],
                        start=True,
                        stop=True,
                    )
                    # bias already folded in via the cvec PSUM init; the
                    # staging copy runs on DVE to rebalance (ACT ~107us vs
                    # DVE ~94us measured)
                    ow = owp.tile([P, WIN], F32)
                    nc.vector.tensor_copy(ow[:, :wan], ops3[:, :wan])
                    nc.scalar.dma_start(out_h[:, wa0 : wa0 + wan], ow[:, :wan])

                gctr = 0  # global group counter for multiply-engine routing
                for s in range(NSW):
                    w_i, sl = divmod(s, WPS)
                    cap = caps[s]
                    off = offs[s]
                    ab = abases[s]
                    xjt = xjp.tile([P, cap, P], FP8)
                    nc.sync.dma_start(
                        xjt[:], xjdev_h[:, off * P : (off + cap) * P].rearrange(
                            "p (c e) -> p c e", e=P
                        )
                    )
                    wt = wp.tile([P, cap, P], BF16)
                    nc.sync.dma_start(
                        wt[:], wdev_h[:, off * P : (off + cap) * P].rearrange(
                            "p (c e) -> p c e", e=P
                        )
                    )
                    # banded one-hot (gpsimd/ACT cannot run TensorTensor, so
                    # DVE it is; the narrow band keeps the 1x cost small);
                    # [P(edge), chunk, atom] stays contiguous for the mm2 rhs
                    oh = ohp.tile([P, cap, WBAND], BF16)
                    nc.vector.tensor_tensor(
                        out=oh[:],
                        in0=segw_t[:, off : off + cap]
                        .unsqueeze(2)
                        .to_broadcast([P, cap, WBAND]),
                        in1=iota_t[:, :cap, :],
                        op=mybir.AluOpType.is_equal,
                    )
                    if sl == 0:
                        psT = ps2.tile([P, WIN], F32)
                        # init the bank to c (x) ones (bias folded through
                        # inv(W_out)); mm2s accumulate (start=False) since
                        # neighboring bands overlap
                        nc.tensor.matmul(
                            psT[:], lhsT=cvec_t[:, :], rhs=ones_t[:, :],
                            start=True, stop=True, skip_group_check=True,
                        )

                    ngrp = (cap + GRP - 1) // GRP
                    grp_ps = {}

                    def emit_m1g(g):
                        r = min(GRP, cap - g * GRP)
                        fj = ps1.tile([P, GRP, P], F32)
                        for i in range(r):
                            nc.tensor.matmul(
                                fj[:, i, :],
                                lhsT=xjt[:, g * GRP + i, :],
                                rhs=win_t[:],
                                start=True,
                                stop=True,
                            )
                        grp_ps[g] = (fj, r)

                    def emit_tail(g):
                        nonlocal gctr
                        fj, r = grp_ps.pop(g)
                        c0 = g * GRP
                        # per-group wf tile: the next group's multiply must
                        # not inherit a WAR dependency on this group's mm2s
                        wf = wfp.tile([P, GRP, P], BF16)
                        # 4/5 of multiplies stage through ACT for the DVE 2x
                        # path; 1/5 read PSUM directly on DVE (balances the
                        # ACT and DVE engine budgets)
                        route = (0, 0, 0, 0, 1)[gctr % 5]
                        gctr += 1
                        if route == 0:
                            fjs = fjp.tile([P, GRP, P], BF16)
                            nc.scalar.copy(fjs[:, :r, :], fj[:, :r, :])
                            nc.vector.tensor_mul(
                                wf[:, :r, :],
                                wt[:, c0 : c0 + r, :],
                                fjs[:, :r, :],
                            )
                        else:
                            nc.vector.tensor_mul(
                                wf[:, :r, :],
                                wt[:, c0 : c0 + r, :],
                                fj[:, :r, :],
                            )
                        for i in range(r):
                            ch = c0 + i
                            a0 = sl * SUB + ab[ch]
                            nc.tensor.matmul(
                                psT[:, a0 : a0 + WBAND],
                                lhsT=wf[:, i, :],
                                rhs=oh[:, ch, :],
                                start=False,
                                stop=True,
                                skip_group_check=True,
                            )

                    for g in range(min(LOOKG, ngrp)):
                        emit_m1g(g)
                    # flush fac2out one sub-window AFTER the window closes:
                    # by then its mm2s have retired, so the ACT-queue cvt
                    # doesn't head-of-line-block the fjs copies behind it
                    if sl == 1 or s == NSW - 1:
                        flush_pending()
                    for g in range(ngrp):
                        if g + LOOKG < ngrp:
                            emit_m1g(g + LOOKG)
                        emit_tail(g)

                    if sl == WPS - 1 or s == NSW - 1:
                        wa0 = w_i * WIN
                        pending = (psT, wa0, min(WIN, APC - wa0))
                flush_pending()
    return nc


def prepare(inputs):
    """Host-side sharding: per-core padded edge buckets in bf16."""
    x = np.ascontiguousarray(np.asarray(inputs["x"], dtype=np.float32))
    w_ij = np.ascontiguousarray(np.asarray(inputs["w_ij"], dtype=np.float32))
    seg_i = np.asarray(inputs["seg_i"]).astype(np.int64).ravel()
    idx_j = np.asarray(inputs["idx_j"]).astype(np.int64).ravel()
    W_in = np.asarray(inputs["W_in"], dtype=np.float32)
    W_out = np.asarray(inputs["W_out"], dtype=np.float32)
    b_out = np.asarray(inputs["b_out"], dtype=np.float32).ravel()

    # edge run boundaries for every 128-atom sub-window of every core
    bounds = np.asarray(
        [c * APC + s * SUB for c in range(NCORES) for s in range(NSW)] + [NA],
        dtype=np.int64,
    )
    edges = np.searchsorted(seg_i, bounds)
    n = (edges[1:] - edges[:-1]).reshape(NCORES, NSW)
    caps = np.maximum(1, -(-n.max(axis=0) // P))  # per-sub-window chunk cap
    offs = np.concatenate([[0], np.cumsum(caps)])
    ctot = int(offs[-1])
    capmax = int(caps.max())

    x_f8 = x.astype(NPFP8)
    w_bf = w_ij.astype(NPBF16)
    # per-(s, ch) narrow-band atom base: union of the chunk's atom range
    # over all 8 cores (seg_i sorted => span is small; measured max 21)
    abases = []
    for s in range(NSW):
        cap = int(caps[s])
        ab = []
        for ch in range(cap):
            lo_u, hi_u = P, -1
            for c in range(NCORES):
                l, h = int(edges[c * NSW + s]), int(edges[c * NSW + s + 1])
                chunk = seg_i[l + ch * P : l + min((ch + 1) * P, h - l)]
                if chunk.size:
                    base = c * APC + s * SUB
                    lo_u = min(lo_u, int(chunk[0] - base))
                    hi_u = max(hi_u, int(chunk[-1] - base))
            if hi_u < 0:
                ab.append(0)
            else:
                assert hi_u - lo_u + 1 <= WBAND, "chunk atom span exceeds WBAND"
                ab.append(max(0, min(lo_u, P - WBAND)))
        abases.append(ab)

    # iota3[p, c, aa] = aa, flattened to [P, capmax*WBAND]
    iota_t = np.ascontiguousarray(
        np.broadcast_to(
            np.tile(np.arange(WBAND, dtype=np.float32), capmax).astype(NPBF16),
            (P, capmax * WBAND),
        )
    )
    win_b = W_in.astype(NPBF16)
    wout_b = W_out.astype(NPBF16)
    # bias folded through inv(W_out): psT init with c makes conv@W_out
    # come out pre-biased
    cvec = np.linalg.solve(
        W_out.astype(np.float64).T, b_out.astype(np.float64)
    ).astype(np.float32)[None, :].astype(NPBF16)

    in_maps = []
    for c in range(NCORES):
        xjdev = np.zeros((P, ctot * P), dtype=NPFP8)
        wdev = np.zeros((P, ctot * P), dtype=NPBF16)
        segw = np.full((P, ctot), -1.0, dtype=NPBF16)
        for s in range(NSW):
            k = c * NSW + s
            lo, hi = int(edges[k]), int(edges[k + 1])
            cnt = hi - lo
            cap = int(caps[s])
            off = int(offs[s])
            xj = np.zeros((cap * P, D), dtype=NPFP8)
            xj[:cnt] = x_f8[idx_j[lo:hi]]
            # lhsT layout [k, (chunk, edge)]
            xjdev[:, off * P : (off + cap) * P] = (
                xj.reshape(cap, P, D).transpose(2, 0, 1).reshape(D, cap * P)
            )
            wpad = np.zeros((cap * P, D), dtype=NPBF16)
            wpad[:cnt] = w_bf[lo:hi]
            # [edge, (chunk, feature)]
            wdev[:, off * P : (off + cap) * P] = (
                wpad.reshape(cap, P, D).transpose(1, 0, 2).reshape(P, cap * P)
            )
            sp = np.full(cap * P, -1.0, dtype=np.float32)
            sp[:cnt] = (seg_i[lo:hi] - (c * APC + s * SUB)).astype(np.float32)
            # rebase each chunk to its narrow-band window
            for ch in range(cap):
                sp[ch * P : (ch + 1) * P] -= np.float32(abases[s][ch])
            sp[cnt:] = -1.0
            segw[:, off : off + cap] = sp.reshape(cap, P).T.astype(NPBF16)
        in_maps.append(
            {
                "xjdev": xjdev,
                "wdev": wdev,
                "segw": segw,
                "iota": iota_t,
                "Win": win_b,
                "Wout": wout_b,
                "cvec": cvec,
            }
        )
    return ([int(c) for c in caps], abases), in_maps


def kernel(**inputs) -> np.ndarray:
    from concourse.bass_utils import run_bass_kernel_spmd

    plan, in_maps = prepare(inputs)
    nc = build_program(plan)
    nc.finalize()
    res = run_bass_kernel_spmd(nc, in_maps, core_ids=list(range(NCORES)))
    outT = np.concatenate([r["out"] for r in res.results], axis=1)
    return np.ascontiguousarray(outT.T)
